# revision 29
# baseline (speedup 1.0000x reference)
import sys

sys.path.insert(0, "/opt/trn_rl_repo")

import numpy as np
import ml_dtypes

from concourse import bass, bacc, tile, bass_utils
from concourse.bass import mybir

F32 = mybir.dt.float32
BF16 = mybir.dt.bfloat16
FP8 = mybir.dt.float8e4
BF = ml_dtypes.bfloat16
E4 = ml_dtypes.float8_e4m3fn

N = 50000
E = 1600000
NG = 64
H = 64
EPS = 1e-5
NCORES = 8
NPC = N // NCORES          # dst-range nodes per core
GROUP_QUADS = 1024         # quads per device group (512 cols x 2 halves)
SLAB_GROUPS = 4            # groups per DMA slab

IN_DT = BF16               # device input dtype (BF16 or FP8)
IN_NP = BF

LAST_EXEC_NS = [0, 0]
LAST_TRACES = {}


# ---------------------------------------------------------------- device ----

def _build(G):
    """One program serves both layers: x2 = W_blkdiag.T @ xin per 512-col tile,
    then quad-max via elementwise max of 4 PSUM tiles (quad edges are split
    across 4 consecutive tiles at the same column)."""
    nc = bacc.Bacc()
    xin = nc.declare_dram_parameter("xin", [128, G * 2048], IN_DT, isOutput=False)
    w = nc.declare_dram_parameter("w", [128, 128], BF16, isOutput=False)
    q = nc.declare_dram_parameter("q", [128, G * 512], BF16, isOutput=True)
    # Ramped slab schedule: small slabs at the start (compute begins early)
    # and at the end (short tail drain).
    sizes = []
    rem = G
    for s in (1, 2):
        if rem > 0:
            sizes.append(min(s, rem))
            rem -= sizes[-1]
    n_full = max(0, (rem - 2) // SLAB_GROUPS)
    sizes += [SLAB_GROUPS] * n_full
    rem -= n_full * SLAB_GROUPS
    while rem > 0:
        s = min(2, rem)
        sizes.append(s)
        rem -= s
    chunks = []
    g = 0
    for s in sizes:
        chunks.append((g, g + s))
        g += s
    with tile.TileContext(nc) as tc:
        with (
            tc.tile_pool(name="c", bufs=1) as cpool,
            tc.tile_pool(name="i", bufs=4) as ipool,
            tc.tile_pool(name="m", bufs=4) as mpool,
            tc.tile_pool(name="o", bufs=5) as opool,
            tc.tile_pool(name="p", bufs=2, space="PSUM") as ppool,
        ):
            wt = cpool.tile([128, 128], BF16)
            nc.sync.dma_start(out=wt[:], in_=w[:])
            # Output flushes are emitted two chunks late so they never
            # head-of-line block the input stream on the sync queue.
            pending = []
            for ci, (g0, g1) in enumerate(chunks):
                ng = g1 - g0
                it = ipool.tile([128, SLAB_GROUPS * 2048], IN_DT)
                if ci == 0:
                    # Split the very first load so the first matmuls wait on
                    # half the bytes.
                    nc.sync.dma_start(out=it[:, :1024],
                                      in_=xin[:, :1024])
                    nc.sync.dma_start(out=it[:, 1024:ng * 2048],
                                      in_=xin[:, 1024:g1 * 2048])
                else:
                    nc.sync.dma_start(out=it[:, :ng * 2048],
                                      in_=xin[:, g0 * 2048:g1 * 2048])
                ot = opool.tile([128, SLAB_GROUPS * 512], BF16)
                vv = None
                for gi in range(ng):
                    # Quad edges live at the same column of 4 consecutive
                    # tiles. DVE can read at most one PSUM operand per op:
                    # ACT stages banks 0-1 into SBUF in one 2-bank copy,
                    # DVE maxes banks 2-3 against them in one 2-bank op.
                    # The final bf16 max is fused across two groups so the
                    # 2x-mode op amortizes its issue cost.
                    psA = ppool.tile([128, 2, 512], F32)
                    psB = ppool.tile([128, 2, 512], F32)
                    for t in range(2):
                        c = (gi * 4 + t) * 512
                        nc.tensor.matmul(psA[:, t, :], wt[:], it[:, c:c + 512],
                                         start=True, stop=True)
                    for t in range(2, 4):
                        c = (gi * 4 + t) * 512
                        nc.tensor.matmul(psB[:, t - 2, :], wt[:],
                                         it[:, c:c + 512],
                                         start=True, stop=True)
                    u01 = mpool.tile([128, 2, 512], BF16)
                    nc.scalar.copy(out=u01[:], in_=psA[:])
                    if gi == 0:
                        vv = mpool.tile([128, ng, 2, 512], BF16)
                    nc.vector.tensor_tensor(out=vv[:, gi, :, :],
                                            in0=psB[:],
                                            in1=u01[:], op=mybir.AluOpType.max)
                # One chunk-wide 2x-mode stage-3 finishes all groups at once.
                nc.vector.tensor_tensor(
                    out=ot[:, :ng * 512].rearrange("p (g f) -> p g f", g=ng),
                    in0=vv[:, :, 0, :], in1=vv[:, :, 1, :],
                    op=mybir.AluOpType.max)
                pending.append((ot, g0, g1))
                if len(pending) > 2:
                    pot, pg0, pg1 = pending.pop(0)
                    nc.sync.dma_start(out=q[:, pg0 * 512:pg1 * 512],
                                      in_=pot[:, :(pg1 - pg0) * 512])
            for pot, pg0, pg1 in pending:
                nc.sync.dma_start(out=q[:, pg0 * 512:pg1 * 512],
                                  in_=pot[:, :(pg1 - pg0) * 512])
    return nc


TRACE = False              # test.py sets True (with its NTFF hook installed)


def _run(nc, in_maps, slot):
    if not nc.is_finalized():
        nc.finalize()
    if TRACE:
        try:
            br = bass_utils.run_bass_kernel_spmd(nc, in_maps,
                                                 list(range(NCORES)),
                                                 trace=True)
        except Exception:
            br = bass_utils.run_bass_kernel_spmd(nc, in_maps,
                                                 list(range(NCORES)),
                                                 trace=False)
    else:
        br = bass_utils.run_bass_kernel_spmd(nc, in_maps, list(range(NCORES)),
                                             trace=False)
    LAST_EXEC_NS[slot] = br.exec_time_ns or 0
    if br.instructions_and_trace:
        LAST_TRACES["L%d" % (slot + 1)] = br.instructions_and_trace[1]
    return br


# ------------------------------------------------------------------ host ----

def _pad_mult4(eids, d):
    """eids: edge ids sorted by dst value d. Pad each dst-run to a multiple of
    4 by duplicating the run's last edge. Returns (padded_eids, nodes, qcnt)."""
    m = eids.shape[0]
    if m == 0:
        z = np.zeros(0, dtype=np.int64)
        return z, z, z
    nodes, counts = np.unique(d, return_counts=True)
    pads = (-counts) % 4
    ends = np.cumsum(counts)
    rep = np.ones(m, dtype=np.int64)
    rep[ends - 1] += pads
    pe = np.repeat(eids, rep)
    qcnt = (counts + pads) // 4
    return pe, nodes, qcnt


def _stats(P2, Q, src, dst):
    """Biased mean/var over edges of x = P2[src] - Q[dst], f64 accumulation."""
    sx = np.zeros(H, np.float64)
    sxx = np.zeros(H, np.float64)
    CH = 262144
    for c0 in range(0, E, CH):
        c1 = min(c0 + CH, E)
        X = P2[src[c0:c1]] - Q[dst[c0:c1]]
        sx += X.sum(0, dtype=np.float64)
        sxx += np.einsum("ij,ij->j", X, X, dtype=np.float64)
    mean = sx / E
    var = sxx / E - mean * mean
    return mean, var


def _make_xin(Pf, Qf, src_pe, dst_pe, G):
    """Materialize relu(Pf[src]-Qf[dst]) for the padded per-core edge stream
    and lay it out [128, G*2048]: quad q's 4 edges at the same column of 4
    consecutive tiles; halves A/B stacked on partitions."""
    X = Pf[src_pe]
    X -= Qf[dst_pe]
    np.maximum(X, 0.0, out=X)
    Xc = X.astype(IN_NP)                      # [NQ*4, 64]
    NQ2 = G * 512                             # quads per half
    halves = Xc.reshape(2, G, 512, 4, H)      # (half, g, col, t, feat)
    xin = np.empty((128, G * 2048), dtype=IN_NP)
    for hh in range(2):
        a = halves[hh].transpose(3, 0, 2, 1)  # [feat, g, t, col]
        xin[hh * H:(hh + 1) * H] = a.reshape(H, G * 2048)
    return np.ascontiguousarray(xin)


def _make_wblk(W2):
    wb = np.zeros((128, 128), dtype=BF)
    wb[0:H, 0:H] = W2.astype(BF)
    wb[H:128, H:128] = W2.astype(BF)
    return wb


def _read_quads(qdev, Qreal):
    """qdev [128, G*512] bf16 -> [Qreal, 64] f32 quad stream."""
    V = np.concatenate([qdev[0:H].T, qdev[H:128].T], axis=0)
    return V[:Qreal].astype(np.float32)


def kernel(**inputs):
    pos = np.asarray(inputs["pos"], dtype=np.float32)
    ei = np.asarray(inputs["edge_index"])
    batch = np.asarray(inputs["batch"])
    W1a = np.asarray(inputs["W1a"], dtype=np.float32)
    b1a = np.asarray(inputs["b1a"], dtype=np.float32)
    g1a = np.asarray(inputs["g1a"], dtype=np.float64)
    be1a = np.asarray(inputs["be1a"], dtype=np.float64)
    W2a = np.asarray(inputs["W2a"], dtype=np.float32)
    b2a = np.asarray(inputs["b2a"], dtype=np.float32)
    W1b = np.asarray(inputs["W1b"], dtype=np.float32)
    b1b = np.asarray(inputs["b1b"], dtype=np.float32)
    g1b = np.asarray(inputs["g1b"], dtype=np.float64)
    be1b = np.asarray(inputs["be1b"], dtype=np.float64)
    W2b = np.asarray(inputs["W2b"], dtype=np.float32)
    b2b = np.asarray(inputs["b2b"], dtype=np.float32)
    Wc = np.asarray(inputs["Wc"], dtype=np.float64)
    bc = np.asarray(inputs["bc"], dtype=np.float64)

    src = ei[0].astype(np.int64)
    dst = ei[1].astype(np.int64)

    ord0 = np.argsort(dst, kind="stable")
    src_s = src[ord0]
    dst_s = dst[ord0]

    # --- shard by dst range; pad runs to x4; uniform group count ---
    shards = []
    for k in range(NCORES):
        lo = np.searchsorted(dst_s, k * NPC, side="left")
        hi = np.searchsorted(dst_s, (k + 1) * NPC, side="left")
        pe, nodes, qcnt = _pad_mult4(ord0[lo:hi], dst_s[lo:hi])
        shards.append((pe, nodes, qcnt))
    Qmax = max(len(s[0]) // 4 for s in shards)
    G = (Qmax + GROUP_QUADS - 1) // GROUP_QUADS

    core_idx = []
    for k in range(NCORES):
        pe = shards[k][0]
        pef = np.zeros(G * GROUP_QUADS * 4, dtype=np.int64)
        pef[:len(pe)] = pe
        core_idx.append((src[pef].astype(np.int32), dst[pef].astype(np.int32)))

    nc = _build(G)
    nc.finalize()

    # ---------------- Layer A ----------------
    W1as = W1a[0:3] + W1a[3:6]
    PA2 = pos @ W1as + b1a
    QA = pos @ W1a[3:6]
    mean_a, var_a = _stats(PA2, QA, src, dst)
    sA = (g1a / np.sqrt(var_a + EPS)).astype(np.float32)
    tA = (be1a - mean_a * (g1a / np.sqrt(var_a + EPS))).astype(np.float32)
    PfA = sA * PA2 + tA
    QfA = sA * QA

    wA = _make_wblk(W2a)
    in_maps = []
    for k in range(NCORES):
        sp, dp = core_idx[k]
        in_maps.append({"xin": _make_xin(PfA, QfA, sp, dp, G), "w": wA})
    br = _run(nc, in_maps, 0)

    h1 = np.zeros((N, H), dtype=np.float32)
    for k in range(NCORES):
        pe, nodes, qcnt = shards[k]
        if len(nodes) == 0:
            continue
        Qreal = int(qcnt.sum())
        V = _read_quads(br.results[k]["q"], Qreal)
        starts = np.zeros(len(qcnt), dtype=np.int64)
        np.cumsum(qcnt[:-1], out=starts[1:])
        agg = np.maximum.reduceat(V, starts, axis=0)
        h1[nodes] = np.maximum(agg + b2a, 0.0)

    # ---------------- Layer B ----------------
    Wt = W1b[64:67]
    PB2 = h1 @ W1b[0:64] + pos @ Wt + b1b
    QB = pos @ Wt
    mean_b, var_b = _stats(PB2, QB, src, dst)
    sB = (g1b / np.sqrt(var_b + EPS)).astype(np.float32)
    tB = (be1b - mean_b * (g1b / np.sqrt(var_b + EPS))).astype(np.float32)
    PfB = sB * PB2 + tB
    QfB = sB * QB

    wB = _make_wblk(W2b)
    in_maps = []
    for k in range(NCORES):
        sp, dp = core_idx[k]
        in_maps.append({"xin": _make_xin(PfB, QfB, sp, dp, G), "w": wB})
    br = _run(nc, in_maps, 1)

    h2 = np.zeros((N, H), dtype=np.float64)
    for k in range(NCORES):
        pe, nodes, qcnt = shards[k]
        if len(nodes) == 0:
            continue
        Qreal = int(qcnt.sum())
        V = _read_quads(br.results[k]["q"], Qreal)
        starts = np.zeros(len(qcnt), dtype=np.int64)
        np.cumsum(qcnt[:-1], out=starts[1:])
        agg = np.maximum.reduceat(V, starts, axis=0)
        h2[nodes] = np.maximum(agg.astype(np.float64) + b2b, 0.0)

    # global max pool over sorted batch, then classifier
    counts = np.bincount(batch, minlength=NG)
    nz = counts > 0
    starts = np.zeros(NG, dtype=np.int64)
    np.cumsum(counts[:-1], out=starts[1:])
    g = np.zeros((NG, H), dtype=np.float64)
    if nz.any():
        g[nz] = np.maximum.reduceat(h2, starts[nz], axis=0)
    out = g @ Wc + bc
    return out.astype(np.float32)


# revision 30
# speedup vs baseline: 1.0466x; 1.0466x over previous
import sys

sys.path.insert(0, "/opt/trn_rl_repo")

import numpy as np
import ml_dtypes

from concourse import bass, bacc, tile, bass_utils
from concourse.bass import mybir

F32 = mybir.dt.float32
BF16 = mybir.dt.bfloat16
FP8 = mybir.dt.float8e4
BF = ml_dtypes.bfloat16
E4 = ml_dtypes.float8_e4m3fn

N = 50000
E = 1600000
NG = 64
H = 64
EPS = 1e-5
NCORES = 8
NPC = N // NCORES          # dst-range nodes per core
GROUP_QUADS = 1024         # quads per device group (512 cols x 2 halves)
SLAB_GROUPS = 4            # groups per DMA slab

IN_DT = BF16               # device input dtype (BF16 or FP8)
IN_NP = BF

LAST_EXEC_NS = [0, 0]
LAST_TRACES = {}


# ---------------------------------------------------------------- device ----

def _build(G):
    """One program serves both layers: x2 = W_blkdiag.T @ xin per 512-col tile,
    then quad-max via elementwise max of 4 PSUM tiles (quad edges are split
    across 4 consecutive tiles at the same column)."""
    nc = bacc.Bacc()
    xin = nc.declare_dram_parameter("xin", [128, G * 2048], IN_DT, isOutput=False)
    w = nc.declare_dram_parameter("w", [128, 128], BF16, isOutput=False)
    q = nc.declare_dram_parameter("q", [128, G * 512], BF16, isOutput=True)
    # Ramped slab schedule: small slabs at the start (compute begins early)
    # and at the end (short tail drain).
    sizes = []
    rem = G
    for s in (1, 2):
        if rem > 0:
            sizes.append(min(s, rem))
            rem -= sizes[-1]
    n_full = max(0, (rem - 2) // SLAB_GROUPS)
    sizes += [SLAB_GROUPS] * n_full
    rem -= n_full * SLAB_GROUPS
    while rem > 0:
        s = min(2, rem)
        sizes.append(s)
        rem -= s
    chunks = []
    g = 0
    for s in sizes:
        chunks.append((g, g + s))
        g += s
    with tile.TileContext(nc) as tc:
        with (
            tc.tile_pool(name="c", bufs=1) as cpool,
            tc.tile_pool(name="i", bufs=4) as ipool,
            tc.tile_pool(name="m", bufs=4) as mpool,
            tc.tile_pool(name="o", bufs=5) as opool,
            tc.tile_pool(name="p", bufs=2, space="PSUM") as ppool,
        ):
            wt = cpool.tile([128, 128], BF16)
            nc.sync.dma_start(out=wt[:], in_=w[:])
            # Output flushes are emitted two chunks late so they never
            # head-of-line block the input stream on the sync queue.
            pending = []
            for ci, (g0, g1) in enumerate(chunks):
                ng = g1 - g0
                it = ipool.tile([128, SLAB_GROUPS * 2048], IN_DT)
                if ci == 0:
                    # Split the very first load so the first matmuls wait on
                    # half the bytes.
                    nc.sync.dma_start(out=it[:, :1024],
                                      in_=xin[:, :1024])
                    nc.sync.dma_start(out=it[:, 1024:ng * 2048],
                                      in_=xin[:, 1024:g1 * 2048])
                else:
                    nc.sync.dma_start(out=it[:, :ng * 2048],
                                      in_=xin[:, g0 * 2048:g1 * 2048])
                ot = opool.tile([128, SLAB_GROUPS * 512], BF16)
                vv = None
                for gi in range(ng):
                    # Quad edges live at the same column of 4 consecutive
                    # tiles. DVE can read at most one PSUM operand per op:
                    # ACT stages banks 0-1 into SBUF in one 2-bank copy,
                    # DVE maxes banks 2-3 against them in one 2-bank op.
                    # The final bf16 max is fused across two groups so the
                    # 2x-mode op amortizes its issue cost.
                    psA = ppool.tile([128, 2, 512], F32)
                    psB = ppool.tile([128, 2, 512], F32)
                    for t in range(2):
                        c = (gi * 4 + t) * 512
                        nc.tensor.matmul(psA[:, t, :], wt[:], it[:, c:c + 512],
                                         start=True, stop=True)
                    for t in range(2, 4):
                        c = (gi * 4 + t) * 512
                        nc.tensor.matmul(psB[:, t - 2, :], wt[:],
                                         it[:, c:c + 512],
                                         start=True, stop=True)
                    u01 = mpool.tile([128, 2, 512], BF16)
                    nc.scalar.copy(out=u01[:], in_=psA[:])
                    if gi % 2 == 0:
                        vv = mpool.tile([128, 2, 2, 512], BF16)
                    nc.vector.tensor_tensor(out=vv[:, gi % 2, :, :],
                                            in0=psB[:],
                                            in1=u01[:], op=mybir.AluOpType.max)
                    if gi % 2 == 1:
                        nc.vector.tensor_tensor(
                            out=ot[:, (gi - 1) * 512:(gi + 1) * 512]
                                .rearrange("p (g f) -> p g f", g=2),
                            in0=vv[:, :, 0, :], in1=vv[:, :, 1, :],
                            op=mybir.AluOpType.max)
                if ng % 2 == 1:
                    nc.vector.tensor_tensor(
                        out=ot[:, (ng - 1) * 512:ng * 512],
                        in0=vv[:, 0, 0, :], in1=vv[:, 0, 1, :],
                        op=mybir.AluOpType.max)
                pending.append((ot, g0, g1))
                if len(pending) > 2:
                    pot, pg0, pg1 = pending.pop(0)
                    nc.sync.dma_start(out=q[:, pg0 * 512:pg1 * 512],
                                      in_=pot[:, :(pg1 - pg0) * 512])
            for pot, pg0, pg1 in pending:
                nc.sync.dma_start(out=q[:, pg0 * 512:pg1 * 512],
                                  in_=pot[:, :(pg1 - pg0) * 512])
    return nc


TRACE = False              # test.py sets True (with its NTFF hook installed)


def _run(nc, in_maps, slot):
    if not nc.is_finalized():
        nc.finalize()
    if TRACE:
        try:
            br = bass_utils.run_bass_kernel_spmd(nc, in_maps,
                                                 list(range(NCORES)),
                                                 trace=True)
        except Exception:
            br = bass_utils.run_bass_kernel_spmd(nc, in_maps,
                                                 list(range(NCORES)),
                                                 trace=False)
    else:
        br = bass_utils.run_bass_kernel_spmd(nc, in_maps, list(range(NCORES)),
                                             trace=False)
    LAST_EXEC_NS[slot] = br.exec_time_ns or 0
    if br.instructions_and_trace:
        LAST_TRACES["L%d" % (slot + 1)] = br.instructions_and_trace[1]
    return br


# ------------------------------------------------------------------ host ----

def _pad_mult4(eids, d):
    """eids: edge ids sorted by dst value d. Pad each dst-run to a multiple of
    4 by duplicating the run's last edge. Returns (padded_eids, nodes, qcnt)."""
    m = eids.shape[0]
    if m == 0:
        z = np.zeros(0, dtype=np.int64)
        return z, z, z
    nodes, counts = np.unique(d, return_counts=True)
    pads = (-counts) % 4
    ends = np.cumsum(counts)
    rep = np.ones(m, dtype=np.int64)
    rep[ends - 1] += pads
    pe = np.repeat(eids, rep)
    qcnt = (counts + pads) // 4
    return pe, nodes, qcnt


def _stats(P2, Q, src, dst):
    """Biased mean/var over edges of x = P2[src] - Q[dst], f64 accumulation."""
    sx = np.zeros(H, np.float64)
    sxx = np.zeros(H, np.float64)
    CH = 262144
    for c0 in range(0, E, CH):
        c1 = min(c0 + CH, E)
        X = P2[src[c0:c1]] - Q[dst[c0:c1]]
        sx += X.sum(0, dtype=np.float64)
        sxx += np.einsum("ij,ij->j", X, X, dtype=np.float64)
    mean = sx / E
    var = sxx / E - mean * mean
    return mean, var


def _make_xin(Pf, Qf, src_pe, dst_pe, G):
    """Materialize relu(Pf[src]-Qf[dst]) for the padded per-core edge stream
    and lay it out [128, G*2048]: quad q's 4 edges at the same column of 4
    consecutive tiles; halves A/B stacked on partitions."""
    X = Pf[src_pe]
    X -= Qf[dst_pe]
    np.maximum(X, 0.0, out=X)
    Xc = X.astype(IN_NP)                      # [NQ*4, 64]
    NQ2 = G * 512                             # quads per half
    halves = Xc.reshape(2, G, 512, 4, H)      # (half, g, col, t, feat)
    xin = np.empty((128, G * 2048), dtype=IN_NP)
    for hh in range(2):
        a = halves[hh].transpose(3, 0, 2, 1)  # [feat, g, t, col]
        xin[hh * H:(hh + 1) * H] = a.reshape(H, G * 2048)
    return np.ascontiguousarray(xin)


def _make_wblk(W2):
    wb = np.zeros((128, 128), dtype=BF)
    wb[0:H, 0:H] = W2.astype(BF)
    wb[H:128, H:128] = W2.astype(BF)
    return wb


def _read_quads(qdev, Qreal):
    """qdev [128, G*512] bf16 -> [Qreal, 64] f32 quad stream."""
    V = np.concatenate([qdev[0:H].T, qdev[H:128].T], axis=0)
    return V[:Qreal].astype(np.float32)


def kernel(**inputs):
    pos = np.asarray(inputs["pos"], dtype=np.float32)
    ei = np.asarray(inputs["edge_index"])
    batch = np.asarray(inputs["batch"])
    W1a = np.asarray(inputs["W1a"], dtype=np.float32)
    b1a = np.asarray(inputs["b1a"], dtype=np.float32)
    g1a = np.asarray(inputs["g1a"], dtype=np.float64)
    be1a = np.asarray(inputs["be1a"], dtype=np.float64)
    W2a = np.asarray(inputs["W2a"], dtype=np.float32)
    b2a = np.asarray(inputs["b2a"], dtype=np.float32)
    W1b = np.asarray(inputs["W1b"], dtype=np.float32)
    b1b = np.asarray(inputs["b1b"], dtype=np.float32)
    g1b = np.asarray(inputs["g1b"], dtype=np.float64)
    be1b = np.asarray(inputs["be1b"], dtype=np.float64)
    W2b = np.asarray(inputs["W2b"], dtype=np.float32)
    b2b = np.asarray(inputs["b2b"], dtype=np.float32)
    Wc = np.asarray(inputs["Wc"], dtype=np.float64)
    bc = np.asarray(inputs["bc"], dtype=np.float64)

    src = ei[0].astype(np.int64)
    dst = ei[1].astype(np.int64)

    ord0 = np.argsort(dst, kind="stable")
    src_s = src[ord0]
    dst_s = dst[ord0]

    # --- shard by dst range; pad runs to x4; uniform group count ---
    shards = []
    for k in range(NCORES):
        lo = np.searchsorted(dst_s, k * NPC, side="left")
        hi = np.searchsorted(dst_s, (k + 1) * NPC, side="left")
        pe, nodes, qcnt = _pad_mult4(ord0[lo:hi], dst_s[lo:hi])
        shards.append((pe, nodes, qcnt))
    Qmax = max(len(s[0]) // 4 for s in shards)
    G = (Qmax + GROUP_QUADS - 1) // GROUP_QUADS

    core_idx = []
    for k in range(NCORES):
        pe = shards[k][0]
        pef = np.zeros(G * GROUP_QUADS * 4, dtype=np.int64)
        pef[:len(pe)] = pe
        core_idx.append((src[pef].astype(np.int32), dst[pef].astype(np.int32)))

    nc = _build(G)
    nc.finalize()

    # ---------------- Layer A ----------------
    W1as = W1a[0:3] + W1a[3:6]
    PA2 = pos @ W1as + b1a
    QA = pos @ W1a[3:6]
    mean_a, var_a = _stats(PA2, QA, src, dst)
    sA = (g1a / np.sqrt(var_a + EPS)).astype(np.float32)
    tA = (be1a - mean_a * (g1a / np.sqrt(var_a + EPS))).astype(np.float32)
    PfA = sA * PA2 + tA
    QfA = sA * QA

    wA = _make_wblk(W2a)
    in_maps = []
    for k in range(NCORES):
        sp, dp = core_idx[k]
        in_maps.append({"xin": _make_xin(PfA, QfA, sp, dp, G), "w": wA})
    br = _run(nc, in_maps, 0)

    h1 = np.zeros((N, H), dtype=np.float32)
    for k in range(NCORES):
        pe, nodes, qcnt = shards[k]
        if len(nodes) == 0:
            continue
        Qreal = int(qcnt.sum())
        V = _read_quads(br.results[k]["q"], Qreal)
        starts = np.zeros(len(qcnt), dtype=np.int64)
        np.cumsum(qcnt[:-1], out=starts[1:])
        agg = np.maximum.reduceat(V, starts, axis=0)
        h1[nodes] = np.maximum(agg + b2a, 0.0)

    # ---------------- Layer B ----------------
    Wt = W1b[64:67]
    PB2 = h1 @ W1b[0:64] + pos @ Wt + b1b
    QB = pos @ Wt
    mean_b, var_b = _stats(PB2, QB, src, dst)
    sB = (g1b / np.sqrt(var_b + EPS)).astype(np.float32)
    tB = (be1b - mean_b * (g1b / np.sqrt(var_b + EPS))).astype(np.float32)
    PfB = sB * PB2 + tB
    QfB = sB * QB

    wB = _make_wblk(W2b)
    in_maps = []
    for k in range(NCORES):
        sp, dp = core_idx[k]
        in_maps.append({"xin": _make_xin(PfB, QfB, sp, dp, G), "w": wB})
    br = _run(nc, in_maps, 1)

    h2 = np.zeros((N, H), dtype=np.float64)
    for k in range(NCORES):
        pe, nodes, qcnt = shards[k]
        if len(nodes) == 0:
            continue
        Qreal = int(qcnt.sum())
        V = _read_quads(br.results[k]["q"], Qreal)
        starts = np.zeros(len(qcnt), dtype=np.int64)
        np.cumsum(qcnt[:-1], out=starts[1:])
        agg = np.maximum.reduceat(V, starts, axis=0)
        h2[nodes] = np.maximum(agg.astype(np.float64) + b2b, 0.0)

    # global max pool over sorted batch, then classifier
    counts = np.bincount(batch, minlength=NG)
    nz = counts > 0
    starts = np.zeros(NG, dtype=np.int64)
    np.cumsum(counts[:-1], out=starts[1:])
    g = np.zeros((NG, H), dtype=np.float64)
    if nz.any():
        g[nz] = np.maximum.reduceat(h2, starts[nz], axis=0)
    out = g @ Wc + bc
    return out.astype(np.float32)
